# revision 21
# baseline (speedup 1.0000x reference)
"""Causal multi-head attention (B=4, S=2048, H=16, D=64, E=1024) on 8 TRN2 cores.

Sharding: data-parallel over batch (4) x tensor-parallel over heads (2 groups
of 8). Each core computes, for its (batch, head-group):
    q/k/v projections -> causal softmax attention -> output projection
and returns a partial [S, E] output (Wr row-split); the host adds the two
partials per batch.

All matmul operands are bf16 (fp32 PSUM accumulation).  Attention runs in the
transposed layout (keys/head-dims on partitions) so no on-chip transposes are
needed; V carries an extra ones-column so the attn@V matmul also emits the
softmax denominators (output row 64).

Pipeline structure (to keep the PE array continuously busy so the HAM clock
gate stays at 2.4 GHz):
  - ladder of 4 rounds: P1(v, 4 seq blocks) -> P2(q/k, chunk r) -> attn(qc=r),
    so exp (ACT engine) overlaps projection matmuls.
  - attention score groups alternate two PSUM buffers (3 and 2 key-blocks);
    attn@V matmuls are deferred 2 groups so the tensor queue never waits on
    the exp.
  - softmax epilogue (reciprocal -> partition broadcast -> scale) runs on
    DVE/GPSIMD only - no tensor-engine involvement.
  - output projection is emitted as per-seq-block units spread between
    attention groups of the next chunk.
"""

from collections import deque

import numpy as np
from ml_dtypes import bfloat16

import concourse.bacc as bacc
import concourse.bass as bass
import concourse.mybir as mybir
import concourse.tile as tile
from concourse.bass_utils import run_bass_kernel_spmd

HEADS = 16
HD = 64
EMB = 1024
B, S = 4, 2048
SCALE = 1.0 / 8.0
NCORES = 8
HPC = HEADS // 2          # heads per core (8)
GW = HPC * HD             # head-group width (512)

F32 = mybir.dt.float32
BF16 = mybir.dt.bfloat16
EXP = mybir.ActivationFunctionType.Exp

NQC = 4                   # query chunks of 512
QW = 512                  # query chunk width
NKB = S // 128            # key blocks of 128 (16)
NEC = EMB // 128          # emb chunks (8)
NSB = S // 128            # seq blocks (16)
DEBUG_OUTT = False


def _groups_for(qc):
    """Alternating 3/2-block score groups for query chunk qc."""
    kbmax = 4 * (qc + 1)
    gs, kb, want3 = [], 0, True
    while kb < kbmax:
        n = min(3 if want3 else 2, kbmax - kb)
        gs.append(("A" if want3 else "B", list(range(kb, kb + n))))
        kb += n
        want3 = not want3
    return gs


def build():
    nc = bacc.Bacc("TRN2", target_bir_lowering=False, debug=False)

    xt_d = nc.dram_tensor("xt", [EMB, S], BF16, kind="ExternalInput")
    # wq/wk pre-swizzled on host to [p, hp, e, n] (contiguous single DMA)
    wq_d = nc.dram_tensor("wq", [128, 4, NEC, 128], BF16, kind="ExternalInput")
    wk_d = nc.dram_tensor("wk", [128, 4, NEC, 128], BF16, kind="ExternalInput")
    wv_d = nc.dram_tensor("wv", [EMB, GW], BF16, kind="ExternalInput")
    wr_d = nc.dram_tensor("wr", [GW, EMB], BF16, kind="ExternalInput")
    # consts: [:,0:128] causal tri mask, [:,128:256] ones, [:,256:640] zeros
    cn_d = nc.dram_tensor("consts", [128, 640], BF16, kind="ExternalInput")
    y_d = nc.dram_tensor("y", [S, EMB], F32, kind="ExternalOutput")
    ot_d = (nc.dram_tensor("ot", [NQC, 128, 4, QW], BF16, kind="ExternalOutput")
            if DEBUG_OUTT else None)
    at_d = (nc.dram_tensor("atd", [2, 128, 3, QW], BF16, kind="ExternalOutput")
            if DEBUG_OUTT else None)
    ac_d = (nc.dram_tensor("acd", [128, QW], F32, kind="ExternalOutput")
            if DEBUG_OUTT else None)
    bc_d = (nc.dram_tensor("bcd", [HD, QW], F32, kind="ExternalOutput")
            if DEBUG_OUTT else None)

    with tile.TileContext(nc) as tc, nc.allow_low_precision(reason="bf16 attn"):
        with (
            tc.tile_pool(name="persist", bufs=1) as pp,
            tc.tile_pool(name="attn", bufs=5) as pa,
            tc.tile_pool(name="outp", bufs=4) as po,
            tc.tile_pool(name="recp", bufs=2) as prc,
            tc.tile_pool(name="nump", bufs=2) as pnum,
            tc.tile_pool(name="bcp", bufs=2) as pbc,
            tc.tile_pool(name="ysb", bufs=2) as pyb,
            tc.tile_pool(name="psum", bufs=1, space="PSUM") as ps,
        ):
            xt = pp.tile([128, NEC, S], BF16, tag="xt")
            wqs = pp.tile([128, 4, NEC, 128], BF16, tag="wq")
            wks = pp.tile([128, 4, NEC, 128], BF16, tag="wk")
            wv = pp.tile([128, NEC, GW], BF16, tag="wv")
            wr = pp.tile([128, 4, EMB], BF16, tag="wr")
            kt = pp.tile([128, 4, S], BF16, tag="kt")
            qt = pp.tile([128, NQC, 4, QW], BF16, tag="qt")
            v = pp.tile([128, NKB, HPC, HD + 1], BF16, tag="v")
            mo = pp.tile([128, 640], BF16, tag="consts")
            mask = mo[:, 0:128]
            zeros = mo[:, 256:640]

            # ---- input DMAs, spread across engine queues ----
            nc.gpsimd.dma_start(mo[:], cn_d.ap())
            nc.gpsimd.memset(v[:, :, :, HD], 1.0)
            nc.gpsimd.dma_start(
                wv[:], wv_d.ap().rearrange("(c p) n -> p c n", p=128))
            # xt loads by seq-quarters: round 0 (P1 sb0-3 + P2 c0) only
            # reads cols 0:512, so it can start after ~1MB instead of 4MB
            def xt_eng(e):
                return nc.sync if e % 2 == 0 else (
                    nc.scalar if e < 6 else nc.gpsimd)

            for e in range(NEC):
                xt_eng(e).dma_start(
                    xt[:, e, 0:QW], xt_d.ap()[e * 128:(e + 1) * 128, 0:QW])
            nc.sync.dma_start(wqs[:], wq_d.ap())
            nc.scalar.dma_start(wks[:], wk_d.ap())
            # rest of xt (cols 512:2048) AFTER the weights so it does not
            # steal HBM bandwidth from the round-0 critical path
            for e in range(NEC):
                nc.sync.dma_start(
                    xt[:, e, QW:], xt_d.ap()[e * 128:(e + 1) * 128, QW:])
            nc.gpsimd.dma_start(
                wr[:], wr_d.ap().rearrange("(c p) n -> p c n", p=128))

            # PSUM tags (8 banks total):
            #   scA [128,3,QW] x1 = 3, scB [128,2,QW] x1 = 2,
            #   acc [128,QW] x2 = 2, y [128,QW] x1 = 1
            def sc_tile(tag):
                if tag == "A":
                    return ps.tile([128, 3, QW], F32, tag="scA", bufs=1, name="scA")
                return ps.tile([128, 2, QW], F32, tag="scB", bufs=1, name="scB")

            proj_tag = ["A"]

            def proj_group(emit_mms, copy_out):
                t = sc_tile(proj_tag[0])[:, 0, :]
                proj_tag[0] = "B" if proj_tag[0] == "A" else "A"
                emit_mms(t)
                copy_out(t)
                group_tick()

            # ---- deferred attn@V + output-projection machinery ----
            pend = deque()      # (at_t, blocks, qc, h, acc)
            p4_units = deque()  # (ready_tick, qc, sbl)
            p4_late = deque()   # (qc, sbl): deferred into qc3's stream
            epi_muls = deque()  # (ready_tick, closure): deferred final scale
            cur_qc = [0]
            tick = [0]
            outts = {}

            def emit_av():
                at_t, blocks, qc, h, acc = pend.popleft()
                kbmax = 4 * (qc + 1)
                for j, kb in enumerate(blocks):
                    nc.tensor.matmul(
                        acc[0:HD + 1, :], v[:, kb, h, :], at_t[:, j, :],
                        start=(kb == 0), stop=(kb == kbmax - 1),
                    )
                if blocks[-1] == kbmax - 1:
                    emit_epilogue(qc, h, acc)

            def emit_epilogue(qc, h, acc):
                hp, ho = h // 2, (h % 2) * HD
                num = pnum.tile([HD, QW], F32, tag="num", name="num")
                nc.vector.tensor_copy(num[:], acc[0:HD, :])
                den = prc.tile([1, QW], F32, tag="den", name="den")
                nc.vector.tensor_copy(den[0:1, :], acc[HD:HD + 1, :])
                rec = prc.tile([1, QW], F32, tag="rec", name="rec")
                nc.vector.reciprocal_approx_fast(rec[0:1, :], den[0:1, :])
                bc = pbc.tile([HD, QW], F32, tag="bc", name="bc")
                nc.gpsimd.partition_broadcast(bc[:], rec[0:1, :])
                nc.vector.tensor_mul(
                    outts[qc][ho:ho + HD, hp, :], num[:], bc[:])
                if h == HPC - 1:
                    if ot_d is not None:
                        nc.sync.dma_start(ot_d.ap()[qc], outts[qc][:])
                    if qc < 3:
                        for sbl in range(4):
                            p4_late.append((qc, sbl))
                    else:
                        for sbl in range(4):
                            p4_units.append((tick[0] + 2, qc, sbl))

            def emit_p4_unit(qc, sbl):
                outtc = outts[qc]
                ysb = pyb.tile([128, EMB], F32, tag="ysb", name="ysb")
                for ncol in range(2):
                    yps = ps.tile([128, QW], F32, tag="y", bufs=1, name="yps")
                    for hp in range(4):
                        nc.tensor.matmul(
                            yps[:],
                            outtc[:, hp, sbl * 128:(sbl + 1) * 128],
                            wr[:, hp, ncol * QW:(ncol + 1) * QW],
                            start=(hp == 0), stop=(hp == 3),
                        )
                    nc.vector.tensor_copy(
                        ysb[:, ncol * QW:(ncol + 1) * QW], yps[:])
                sb = qc * 4 + sbl
                nc.sync.dma_start(y_d.ap()[sb * 128:(sb + 1) * 128, :], ysb[:])

            def group_tick():
                tick[0] += 1
                while epi_muls and epi_muls[0][0] <= tick[0]:
                    epi_muls.popleft()[1]()
                if len(pend) >= 4:
                    emit_av()
                if cur_qc[0] == 3 and p4_late and tick[0] % 4 == 0:
                    qc, sbl = p4_late.popleft()
                    emit_p4_unit(qc, sbl)
                if p4_units and p4_units[0][0] <= tick[0]:
                    _, qc, sbl = p4_units.popleft()
                    emit_p4_unit(qc, sbl)

            # ---- projection work units ----
            def emit_p1_unit(sb):
                def mms(t):
                    for e in range(NEC):
                        nc.tensor.matmul(
                            t, xt[:, e, sb * 128:(sb + 1) * 128], wv[:, e, :],
                            start=(e == 0), stop=(e == NEC - 1),
                        )

                def cpy(t):
                    nc.vector.tensor_copy(
                        v[:, sb, :, 0:HD],
                        t.rearrange("p (h d) -> p h d", d=HD),
                    )
                proj_group(mms, cpy)

            def emit_p2_unit(c, wsrc, is_q, hp):
                csl = slice(c * QW, (c + 1) * QW)

                def mms(t):
                    for e in range(NEC):
                        nc.tensor.matmul(
                            t, wsrc[:, hp, e, :], xt[:, e, csl],
                            start=(e == 0), stop=(e == NEC - 1),
                        )

                def cpy(t):
                    if is_q:
                        nc.vector.tensor_copy(qt[:, c, hp, :], t)
                    else:
                        nc.vector.tensor_copy(kt[:, hp, csl], t)
                proj_group(mms, cpy)

            def proj_round_units(rnd):
                units = []
                p1_sbs = range(4 * rnd, 4 * rnd + 4)
                for sb in p1_sbs:
                    units.append(lambda sb=sb: emit_p1_unit(sb))
                for wsrc, is_q in ((wqs, True), (wks, False)):
                    for hp in range(4):
                        units.append(
                            lambda wsrc=wsrc, is_q=is_q, hp=hp:
                            emit_p2_unit(rnd, wsrc, is_q, hp))
                return units

            # round 0: stream P1 (v for sb 0-7) over arriving xt chunks using
            # all 8 PSUM banks as concurrent accumulators, then P2(c=0).
            tA = ps.tile([128, 3, QW], F32, tag="scA", bufs=1, name="scA")
            tB = ps.tile([128, 2, QW], F32, tag="scB", bufs=1, name="scB")
            accs0 = [tA[:, 0, :], tA[:, 1, :], tA[:, 2, :], tB[:, 0, :]]
            for e in range(NEC):
                for i, sb in enumerate(range(4)):
                    nc.tensor.matmul(
                        accs0[i], xt[:, e, sb * 128:(sb + 1) * 128],
                        wv[:, e, :],
                        start=(e == 0), stop=(e == NEC - 1),
                    )
            for i, sb in enumerate(range(4)):
                nc.vector.tensor_copy(
                    v[:, sb, :, 0:HD],
                    accs0[i].rearrange("p (h d) -> p h d", d=HD),
                )
            for u in proj_round_units(0)[4:]:
                u()
            PUNITS_PER_HEAD = [2, 2, 2, 2, 1, 1, 1, 1]

            for rnd in range(4):
                # attention for query chunk qc = rnd
                qc = rnd
                cur_qc[0] = qc
                punits = deque(proj_round_units(rnd + 1)) if rnd < 3 else deque()
                outts[qc] = po.tile([128, 4, QW], BF16, tag="outt", name="outt")
                for h in range(HPC):
                    hp, ho = h // 2, (h % 2) * HD
                    acc = ps.tile([128, QW], F32, tag="acc", bufs=2, name="acc")
                    for gtag, blocks in _groups_for(qc):
                        n = len(blocks)
                        sc_t = sc_tile(gtag)
                        for j, kb in enumerate(blocks):
                            q0 = max(0, (kb - 4 * qc)) * 128
                            nc.tensor.matmul(
                                sc_t[:, j, q0:],
                                kt[ho:ho + HD, hp, kb * 128:(kb + 1) * 128],
                                qt[ho:ho + HD, qc, hp, q0:],
                                start=True, stop=True,
                            )
                        at_t = pa.tile([128, 3, QW], BF16, tag="at", name="at")
                        nc.scalar.activation(
                            at_t[:, 0:n, :], sc_t[:, 0:n, :], EXP)
                        for j, kb in enumerate(blocks):
                            jj = kb - 4 * qc
                            if jj >= 0:  # diagonal block: causal mask
                                if jj > 0:
                                    nc.vector.tensor_copy(
                                        at_t[:, j, 0:jj * 128],
                                        zeros[:, 0:jj * 128])
                                nc.vector.tensor_mul(
                                    at_t[:, j, jj * 128:(jj + 1) * 128],
                                    at_t[:, j, jj * 128:(jj + 1) * 128],
                                    mask,
                                )
                        if ot_d is not None and qc == 0 and h == 0:
                            gi = 0 if blocks[0] == 0 else 1
                            nc.sync.dma_start(at_d.ap()[gi], at_t[:])
                        pend.append((at_t, blocks, qc, h, acc))
                        group_tick()
                    for _ in range(PUNITS_PER_HEAD[h]):
                        if punits:
                            punits.popleft()()
                    if h == HPC - 1:
                        while punits:
                            punits.popleft()()

            # ---- drain ----
            while pend:
                emit_av()
                tick[0] += 1
                while epi_muls and epi_muls[0][0] <= tick[0]:
                    epi_muls.popleft()[1]()
                if p4_late:
                    qc, sbl = p4_late.popleft()
                    emit_p4_unit(qc, sbl)
            while epi_muls:
                epi_muls.popleft()[1]()
            while p4_late:
                qc, sbl = p4_late.popleft()
                emit_p4_unit(qc, sbl)
            while p4_units:
                _, qc, sbl = p4_units.popleft()
                emit_p4_unit(qc, sbl)

    nc.compile()
    return nc


_NC_CACHE = None


def _get_nc():
    global _NC_CACHE
    if _NC_CACHE is None:
        _NC_CACHE = build()
    return _NC_CACHE


def make_in_maps(x, Wq, Wk, Wv, Wr):
    x = np.asarray(x, dtype=np.float32)
    Wq = np.asarray(Wq, dtype=np.float32)
    Wk = np.asarray(Wk, dtype=np.float32)
    Wv = np.asarray(Wv, dtype=np.float32)
    Wr = np.asarray(Wr, dtype=np.float32)

    consts = np.zeros((128, 640), dtype=np.float32)
    consts[:, 0:128] = np.triu(np.ones((128, 128), dtype=np.float32))
    consts[:, 128:256] = 1.0
    consts = consts.astype(bfloat16)

    def swz(w):  # [1024, 512] -> [p, hp, e, n]
        return np.ascontiguousarray(
            w.reshape(NEC, 128, 4, 128).transpose(1, 2, 0, 3).astype(bfloat16))

    in_maps = []
    for core in range(NCORES):
        b, g = divmod(core, 2)
        hs = slice(g * GW, (g + 1) * GW)
        in_maps.append({
            "xt": np.ascontiguousarray(x[b].T.astype(bfloat16)),
            "wq": swz(Wq[:, hs] * SCALE),
            "wk": swz(Wk[:, hs]),
            "wv": np.ascontiguousarray(Wv[:, hs].astype(bfloat16)),
            "wr": np.ascontiguousarray(Wr[hs, :].astype(bfloat16)),
            "consts": consts,
        })
    return in_maps


def kernel(x, Wq, Wk, Wv, Wr):
    in_maps = make_in_maps(x, Wq, Wk, Wv, Wr)
    nc = _get_nc()
    res = run_bass_kernel_spmd(nc, in_maps, core_ids=list(range(NCORES)))

    y = np.empty((B, S, EMB), dtype=np.float32)
    for b in range(B):
        y[b] = res.results[2 * b]["y"] + res.results[2 * b + 1]["y"]
    return y


# revision 22
# speedup vs baseline: 1.0087x; 1.0087x over previous
"""Causal multi-head attention (B=4, S=2048, H=16, D=64, E=1024) on 8 TRN2 cores.

Sharding: data-parallel over batch (4) x tensor-parallel over heads (2 groups
of 8). Each core computes, for its (batch, head-group):
    q/k/v projections -> causal softmax attention -> output projection
and returns a partial [S, E] output (Wr row-split); the host adds the two
partials per batch.

All matmul operands are bf16 (fp32 PSUM accumulation).  Attention runs in the
transposed layout (keys/head-dims on partitions) so no on-chip transposes are
needed; V carries an extra ones-column so the attn@V matmul also emits the
softmax denominators (output row 64).

Pipeline structure (to keep the PE array continuously busy so the HAM clock
gate stays at 2.4 GHz):
  - ladder of 4 rounds: P1(v, 4 seq blocks) -> P2(q/k, chunk r) -> attn(qc=r),
    so exp (ACT engine) overlaps projection matmuls.
  - attention score groups alternate two PSUM buffers (3 and 2 key-blocks);
    attn@V matmuls are deferred 2 groups so the tensor queue never waits on
    the exp.
  - softmax epilogue (reciprocal -> partition broadcast -> scale) runs on
    DVE/GPSIMD only - no tensor-engine involvement.
  - output projection is emitted as per-seq-block units spread between
    attention groups of the next chunk.
"""

from collections import deque

import numpy as np
from ml_dtypes import bfloat16

import concourse.bacc as bacc
import concourse.bass as bass
import concourse.mybir as mybir
import concourse.tile as tile
from concourse.bass_utils import run_bass_kernel_spmd

HEADS = 16
HD = 64
EMB = 1024
B, S = 4, 2048
SCALE = 1.0 / 8.0
NCORES = 8
HPC = HEADS // 2          # heads per core (8)
GW = HPC * HD             # head-group width (512)

F32 = mybir.dt.float32
BF16 = mybir.dt.bfloat16
EXP = mybir.ActivationFunctionType.Exp

NQC = 4                   # query chunks of 512
QW = 512                  # query chunk width
NKB = S // 128            # key blocks of 128 (16)
NEC = EMB // 128          # emb chunks (8)
NSB = S // 128            # seq blocks (16)
DEBUG_OUTT = False


def _groups_for(qc):
    """Alternating 3/2-block score groups for query chunk qc."""
    kbmax = 4 * (qc + 1)
    gs, kb, want3 = [], 0, True
    while kb < kbmax:
        n = min(3 if want3 else 2, kbmax - kb)
        gs.append(("A" if want3 else "B", list(range(kb, kb + n))))
        kb += n
        want3 = not want3
    return gs


def build():
    nc = bacc.Bacc("TRN2", target_bir_lowering=False, debug=False)

    xt_d = nc.dram_tensor("xt", [EMB, S], BF16, kind="ExternalInput")
    # wq/wk pre-swizzled on host to [p, hp, e, n] (contiguous single DMA)
    wq_d = nc.dram_tensor("wq", [128, 4, NEC, 128], BF16, kind="ExternalInput")
    wk_d = nc.dram_tensor("wk", [128, 4, NEC, 128], BF16, kind="ExternalInput")
    wv_d = nc.dram_tensor("wv", [128, NEC * GW], BF16, kind="ExternalInput")
    wr_d = nc.dram_tensor("wr", [128, 4 * EMB], BF16, kind="ExternalInput")
    # consts: [:,0:128] causal tri mask, [:,128:256] ones, [:,256:640] zeros
    cn_d = nc.dram_tensor("consts", [128, 640], BF16, kind="ExternalInput")
    y_d = nc.dram_tensor("y", [S, EMB], F32, kind="ExternalOutput")
    ot_d = (nc.dram_tensor("ot", [NQC, 128, 4, QW], BF16, kind="ExternalOutput")
            if DEBUG_OUTT else None)
    at_d = (nc.dram_tensor("atd", [2, 128, 3, QW], BF16, kind="ExternalOutput")
            if DEBUG_OUTT else None)
    ac_d = (nc.dram_tensor("acd", [128, QW], F32, kind="ExternalOutput")
            if DEBUG_OUTT else None)
    bc_d = (nc.dram_tensor("bcd", [HD, QW], F32, kind="ExternalOutput")
            if DEBUG_OUTT else None)

    with tile.TileContext(nc) as tc, nc.allow_low_precision(reason="bf16 attn"):
        with (
            tc.tile_pool(name="persist", bufs=1) as pp,
            tc.tile_pool(name="attn", bufs=5) as pa,
            tc.tile_pool(name="outp", bufs=4) as po,
            tc.tile_pool(name="recp", bufs=2) as prc,
            tc.tile_pool(name="nump", bufs=2) as pnum,
            tc.tile_pool(name="bcp", bufs=2) as pbc,
            tc.tile_pool(name="ysb", bufs=2) as pyb,
            tc.tile_pool(name="psum", bufs=1, space="PSUM") as ps,
        ):
            xt = pp.tile([128, NEC, S], BF16, tag="xt")
            wqs = pp.tile([128, 4, NEC, 128], BF16, tag="wq")
            wks = pp.tile([128, 4, NEC, 128], BF16, tag="wk")
            wv = pp.tile([128, NEC, GW], BF16, tag="wv")
            wr = pp.tile([128, 4, EMB], BF16, tag="wr")
            kt = pp.tile([128, 4, S], BF16, tag="kt")
            qt = pp.tile([128, NQC, 4, QW], BF16, tag="qt")
            v = pp.tile([128, NKB, HPC, HD + 1], BF16, tag="v")
            mo = pp.tile([128, 640], BF16, tag="consts")
            mask = mo[:, 0:128]
            zeros = mo[:, 256:640]

            # ---- input DMAs, spread across engine queues ----
            nc.gpsimd.dma_start(
                wv[:], wv_d.ap().rearrange("p (c n) -> p c n", c=NEC))
            nc.gpsimd.dma_start(mo[:], cn_d.ap())
            nc.gpsimd.memset(v[:, :, :, HD], 1.0)
            # xt loads by seq-quarters: round 0 (P1 sb0-3 + P2 c0) only
            # reads cols 0:512, so it can start after ~1MB instead of 4MB
            def xt_eng(e):
                return nc.sync if e % 2 == 0 else (
                    nc.scalar if e < 6 else nc.gpsimd)

            for e in range(NEC):
                xt_eng(e).dma_start(
                    xt[:, e, 0:QW], xt_d.ap()[e * 128:(e + 1) * 128, 0:QW])
            nc.sync.dma_start(wqs[:], wq_d.ap())
            nc.scalar.dma_start(wks[:], wk_d.ap())
            # rest of xt (cols 512:2048) AFTER the weights so it does not
            # steal HBM bandwidth from the round-0 critical path
            for e in range(NEC):
                nc.sync.dma_start(
                    xt[:, e, QW:], xt_d.ap()[e * 128:(e + 1) * 128, QW:])
            nc.gpsimd.dma_start(
                wr[:], wr_d.ap().rearrange("p (c n) -> p c n", c=4))

            # PSUM tags (8 banks total):
            #   scA [128,3,QW] x1 = 3, scB [128,2,QW] x1 = 2,
            #   acc [128,QW] x2 = 2, y [128,QW] x1 = 1
            def sc_tile(tag):
                if tag == "A":
                    return ps.tile([128, 3, QW], F32, tag="scA", bufs=1, name="scA")
                return ps.tile([128, 2, QW], F32, tag="scB", bufs=1, name="scB")

            proj_tag = ["A"]

            def proj_group(emit_mms, copy_out):
                t = sc_tile(proj_tag[0])[:, 0, :]
                proj_tag[0] = "B" if proj_tag[0] == "A" else "A"
                emit_mms(t)
                copy_out(t)
                group_tick()

            # ---- deferred attn@V + output-projection machinery ----
            pend = deque()      # (at_t, blocks, qc, h, acc)
            p4_units = deque()  # (ready_tick, qc, sbl)
            p4_late = deque()   # (qc, sbl): deferred into qc3's stream
            epi_muls = deque()  # (ready_tick, closure): deferred final scale
            cur_qc = [0]
            tick = [0]
            outts = {}

            def emit_av():
                at_t, blocks, qc, h, acc = pend.popleft()
                kbmax = 4 * (qc + 1)
                for j, kb in enumerate(blocks):
                    nc.tensor.matmul(
                        acc[0:HD + 1, :], v[:, kb, h, :], at_t[:, j, :],
                        start=(kb == 0), stop=(kb == kbmax - 1),
                    )
                if blocks[-1] == kbmax - 1:
                    emit_epilogue(qc, h, acc)

            def emit_epilogue(qc, h, acc):
                hp, ho = h // 2, (h % 2) * HD
                num = pnum.tile([HD, QW], F32, tag="num", name="num")
                nc.vector.tensor_copy(num[:], acc[0:HD, :])
                den = prc.tile([1, QW], F32, tag="den", name="den")
                nc.vector.tensor_copy(den[0:1, :], acc[HD:HD + 1, :])
                rec = prc.tile([1, QW], F32, tag="rec", name="rec")
                nc.vector.reciprocal_approx_fast(rec[0:1, :], den[0:1, :])
                bc = pbc.tile([HD, QW], F32, tag="bc", name="bc")
                nc.gpsimd.partition_broadcast(bc[:], rec[0:1, :])
                nc.vector.tensor_mul(
                    outts[qc][ho:ho + HD, hp, :], num[:], bc[:])
                if h == HPC - 1:
                    if ot_d is not None:
                        nc.sync.dma_start(ot_d.ap()[qc], outts[qc][:])
                    if qc < 3:
                        for sbl in range(4):
                            p4_late.append((qc, sbl))
                    else:
                        for sbl in range(4):
                            p4_units.append((tick[0] + 2, qc, sbl))

            def emit_p4_unit(qc, sbl):
                outtc = outts[qc]
                ysb = pyb.tile([128, EMB], F32, tag="ysb", name="ysb")
                for ncol in range(2):
                    yps = ps.tile([128, QW], F32, tag="y", bufs=1, name="yps")
                    for hp in range(4):
                        nc.tensor.matmul(
                            yps[:],
                            outtc[:, hp, sbl * 128:(sbl + 1) * 128],
                            wr[:, hp, ncol * QW:(ncol + 1) * QW],
                            start=(hp == 0), stop=(hp == 3),
                        )
                    nc.vector.tensor_copy(
                        ysb[:, ncol * QW:(ncol + 1) * QW], yps[:])
                sb = qc * 4 + sbl
                nc.sync.dma_start(y_d.ap()[sb * 128:(sb + 1) * 128, :], ysb[:])

            def group_tick():
                tick[0] += 1
                while epi_muls and epi_muls[0][0] <= tick[0]:
                    epi_muls.popleft()[1]()
                if len(pend) >= 4:
                    emit_av()
                if cur_qc[0] == 3 and p4_late and tick[0] % 4 == 0:
                    qc, sbl = p4_late.popleft()
                    emit_p4_unit(qc, sbl)
                if p4_units and p4_units[0][0] <= tick[0]:
                    _, qc, sbl = p4_units.popleft()
                    emit_p4_unit(qc, sbl)

            # ---- projection work units ----
            def emit_p1_unit(sb):
                def mms(t):
                    for e in range(NEC):
                        nc.tensor.matmul(
                            t, xt[:, e, sb * 128:(sb + 1) * 128], wv[:, e, :],
                            start=(e == 0), stop=(e == NEC - 1),
                        )

                def cpy(t):
                    nc.vector.tensor_copy(
                        v[:, sb, :, 0:HD],
                        t.rearrange("p (h d) -> p h d", d=HD),
                    )
                proj_group(mms, cpy)

            def emit_p2_unit(c, wsrc, is_q, hp):
                csl = slice(c * QW, (c + 1) * QW)

                def mms(t):
                    for e in range(NEC):
                        nc.tensor.matmul(
                            t, wsrc[:, hp, e, :], xt[:, e, csl],
                            start=(e == 0), stop=(e == NEC - 1),
                        )

                def cpy(t):
                    if is_q:
                        nc.vector.tensor_copy(qt[:, c, hp, :], t)
                    else:
                        nc.vector.tensor_copy(kt[:, hp, csl], t)
                proj_group(mms, cpy)

            def proj_round_units(rnd):
                units = []
                p1_sbs = range(4 * rnd, 4 * rnd + 4)
                for sb in p1_sbs:
                    units.append(lambda sb=sb: emit_p1_unit(sb))
                for wsrc, is_q in ((wqs, True), (wks, False)):
                    for hp in range(4):
                        units.append(
                            lambda wsrc=wsrc, is_q=is_q, hp=hp:
                            emit_p2_unit(rnd, wsrc, is_q, hp))
                return units

            # round 0: stream P1 (v for sb 0-7) over arriving xt chunks using
            # all 8 PSUM banks as concurrent accumulators, then P2(c=0).
            tA = ps.tile([128, 3, QW], F32, tag="scA", bufs=1, name="scA")
            tB = ps.tile([128, 2, QW], F32, tag="scB", bufs=1, name="scB")
            accs0 = [tA[:, 0, :], tA[:, 1, :], tA[:, 2, :], tB[:, 0, :]]
            for e in range(NEC):
                for i, sb in enumerate(range(4)):
                    nc.tensor.matmul(
                        accs0[i], xt[:, e, sb * 128:(sb + 1) * 128],
                        wv[:, e, :],
                        start=(e == 0), stop=(e == NEC - 1),
                    )
            for i, sb in enumerate(range(4)):
                nc.vector.tensor_copy(
                    v[:, sb, :, 0:HD],
                    accs0[i].rearrange("p (h d) -> p h d", d=HD),
                )
            for u in proj_round_units(0)[4:]:
                u()
            PUNITS_PER_HEAD = [2, 2, 2, 2, 1, 1, 1, 1]

            for rnd in range(4):
                # attention for query chunk qc = rnd
                qc = rnd
                cur_qc[0] = qc
                punits = deque(proj_round_units(rnd + 1)) if rnd < 3 else deque()
                outts[qc] = po.tile([128, 4, QW], BF16, tag="outt", name="outt")
                for h in range(HPC):
                    hp, ho = h // 2, (h % 2) * HD
                    acc = ps.tile([128, QW], F32, tag="acc", bufs=2, name="acc")
                    for gtag, blocks in _groups_for(qc):
                        n = len(blocks)
                        sc_t = sc_tile(gtag)
                        for j, kb in enumerate(blocks):
                            q0 = max(0, (kb - 4 * qc)) * 128
                            nc.tensor.matmul(
                                sc_t[:, j, q0:],
                                kt[ho:ho + HD, hp, kb * 128:(kb + 1) * 128],
                                qt[ho:ho + HD, qc, hp, q0:],
                                start=True, stop=True,
                            )
                        at_t = pa.tile([128, 3, QW], BF16, tag="at", name="at")
                        nc.scalar.activation(
                            at_t[:, 0:n, :], sc_t[:, 0:n, :], EXP)
                        for j, kb in enumerate(blocks):
                            jj = kb - 4 * qc
                            if jj >= 0:  # diagonal block: causal mask
                                if jj > 0:
                                    nc.vector.tensor_copy(
                                        at_t[:, j, 0:jj * 128],
                                        zeros[:, 0:jj * 128])
                                nc.vector.tensor_mul(
                                    at_t[:, j, jj * 128:(jj + 1) * 128],
                                    at_t[:, j, jj * 128:(jj + 1) * 128],
                                    mask,
                                )
                        if ot_d is not None and qc == 0 and h == 0:
                            gi = 0 if blocks[0] == 0 else 1
                            nc.sync.dma_start(at_d.ap()[gi], at_t[:])
                        pend.append((at_t, blocks, qc, h, acc))
                        group_tick()
                    for _ in range(PUNITS_PER_HEAD[h]):
                        if punits:
                            punits.popleft()()
                    if h == HPC - 1:
                        while punits:
                            punits.popleft()()

            # ---- drain ----
            while pend:
                emit_av()
                tick[0] += 1
                while epi_muls and epi_muls[0][0] <= tick[0]:
                    epi_muls.popleft()[1]()
                if p4_late:
                    qc, sbl = p4_late.popleft()
                    emit_p4_unit(qc, sbl)
            while epi_muls:
                epi_muls.popleft()[1]()
            while p4_late:
                qc, sbl = p4_late.popleft()
                emit_p4_unit(qc, sbl)
            while p4_units:
                _, qc, sbl = p4_units.popleft()
                emit_p4_unit(qc, sbl)

    nc.compile()
    return nc


_NC_CACHE = None


def _get_nc():
    global _NC_CACHE
    if _NC_CACHE is None:
        _NC_CACHE = build()
    return _NC_CACHE


def make_in_maps(x, Wq, Wk, Wv, Wr):
    x = np.asarray(x, dtype=np.float32)
    Wq = np.asarray(Wq, dtype=np.float32)
    Wk = np.asarray(Wk, dtype=np.float32)
    Wv = np.asarray(Wv, dtype=np.float32)
    Wr = np.asarray(Wr, dtype=np.float32)

    consts = np.zeros((128, 640), dtype=np.float32)
    consts[:, 0:128] = np.triu(np.ones((128, 128), dtype=np.float32))
    consts[:, 128:256] = 1.0
    consts = consts.astype(bfloat16)

    def swz(w):  # [1024, 512] -> [p, hp, e, n]
        return np.ascontiguousarray(
            w.reshape(NEC, 128, 4, 128).transpose(1, 2, 0, 3).astype(bfloat16))

    in_maps = []
    for core in range(NCORES):
        b, g = divmod(core, 2)
        hs = slice(g * GW, (g + 1) * GW)
        in_maps.append({
            "xt": np.ascontiguousarray(x[b].T.astype(bfloat16)),
            "wq": swz(Wq[:, hs] * SCALE),
            "wk": swz(Wk[:, hs]),
            "wv": np.ascontiguousarray(
                Wv[:, hs].reshape(NEC, 128, GW).transpose(1, 0, 2)
                .reshape(128, NEC * GW).astype(bfloat16)),
            "wr": np.ascontiguousarray(
                Wr[hs, :].reshape(4, 128, EMB).transpose(1, 0, 2)
                .reshape(128, 4 * EMB).astype(bfloat16)),
            "consts": consts,
        })
    return in_maps


def kernel(x, Wq, Wk, Wv, Wr):
    in_maps = make_in_maps(x, Wq, Wk, Wv, Wr)
    nc = _get_nc()
    res = run_bass_kernel_spmd(nc, in_maps, core_ids=list(range(NCORES)))

    y = np.empty((B, S, EMB), dtype=np.float32)
    for b in range(B):
        y[b] = res.results[2 * b]["y"] + res.results[2 * b + 1]["y"]
    return y


# revision 23
# speedup vs baseline: 1.0720x; 1.0628x over previous
"""Causal multi-head attention (B=4, S=2048, H=16, D=64, E=1024) on 8 TRN2 cores.

Sharding: data-parallel over batch (4) x tensor-parallel over heads (2 groups
of 8). Each core computes, for its (batch, head-group):
    q/k/v projections -> causal softmax attention -> output projection
and returns a partial [S, E] output (Wr row-split); the host adds the two
partials per batch.

All matmul operands are bf16 (fp32 PSUM accumulation).  Attention runs in the
transposed layout (keys/head-dims on partitions) so no on-chip transposes are
needed; V carries an extra ones-column so the attn@V matmul also emits the
softmax denominators (output row 64).

Pipeline structure (to keep the PE array continuously busy so the HAM clock
gate stays at 2.4 GHz):
  - ladder of 4 rounds: P1(v, 4 seq blocks) -> P2(q/k, chunk r) -> attn(qc=r),
    so exp (ACT engine) overlaps projection matmuls.
  - attention score groups alternate two PSUM buffers (3 and 2 key-blocks);
    attn@V matmuls are deferred 2 groups so the tensor queue never waits on
    the exp.
  - softmax epilogue (reciprocal -> partition broadcast -> scale) runs on
    DVE/GPSIMD only - no tensor-engine involvement.
  - output projection is emitted as per-seq-block units spread between
    attention groups of the next chunk.
"""

from collections import deque

import numpy as np
from ml_dtypes import bfloat16

import concourse.bacc as bacc
import concourse.bass as bass
import concourse.mybir as mybir
import concourse.tile as tile
from concourse.bass_utils import run_bass_kernel_spmd

HEADS = 16
HD = 64
EMB = 1024
B, S = 4, 2048
SCALE = 1.0 / 8.0
NCORES = 8
HPC = HEADS // 2          # heads per core (8)
GW = HPC * HD             # head-group width (512)

F32 = mybir.dt.float32
BF16 = mybir.dt.bfloat16
EXP = mybir.ActivationFunctionType.Exp

NQC = 4                   # query chunks of 512
QW = 512                  # query chunk width
NKB = S // 128            # key blocks of 128 (16)
NEC = EMB // 128          # emb chunks (8)
NSB = S // 128            # seq blocks (16)
DEBUG_OUTT = False


def _groups_for(qc):
    """Alternating 3/2-block score groups for query chunk qc."""
    kbmax = 4 * (qc + 1)
    gs, kb, want3 = [], 0, True
    while kb < kbmax:
        n = min(3 if want3 else 2, kbmax - kb)
        gs.append(("A" if want3 else "B", list(range(kb, kb + n))))
        kb += n
        want3 = not want3
    return gs


def build():
    nc = bacc.Bacc("TRN2", target_bir_lowering=False, debug=False)

    xt_d = nc.dram_tensor("xt", [EMB, S], BF16, kind="ExternalInput")
    # wq/wk pre-swizzled on host to [p, hp, e, n] (contiguous single DMA)
    wq_d = nc.dram_tensor("wq", [128, 4, NEC, 128], BF16, kind="ExternalInput")
    wk_d = nc.dram_tensor("wk", [128, 4, NEC, 128], BF16, kind="ExternalInput")
    wv_d = nc.dram_tensor("wv", [128, NEC * GW], BF16, kind="ExternalInput")
    wr_d = nc.dram_tensor("wr", [128, 4 * EMB], BF16, kind="ExternalInput")
    # consts: [:,0:128] causal tri mask, [:,128:256] ones, [:,256:640] zeros
    cn_d = nc.dram_tensor("consts", [128, 640], BF16, kind="ExternalInput")
    y_d = nc.dram_tensor("y", [S, EMB], F32, kind="ExternalOutput")
    ot_d = (nc.dram_tensor("ot", [NQC, 128, 4, QW], BF16, kind="ExternalOutput")
            if DEBUG_OUTT else None)
    at_d = (nc.dram_tensor("atd", [2, 128, 3, QW], BF16, kind="ExternalOutput")
            if DEBUG_OUTT else None)
    ac_d = (nc.dram_tensor("acd", [128, QW], F32, kind="ExternalOutput")
            if DEBUG_OUTT else None)
    bc_d = (nc.dram_tensor("bcd", [HD, QW], F32, kind="ExternalOutput")
            if DEBUG_OUTT else None)

    with tile.TileContext(nc) as tc, nc.allow_low_precision(reason="bf16 attn"):
        with (
            tc.tile_pool(name="persist", bufs=1) as pp,
            tc.tile_pool(name="attn", bufs=5) as pa,
            tc.tile_pool(name="outp", bufs=4) as po,
            tc.tile_pool(name="recp", bufs=2) as prc,
            tc.tile_pool(name="nump", bufs=2) as pnum,
            tc.tile_pool(name="bcp", bufs=2) as pbc,
            tc.tile_pool(name="ysb", bufs=2) as pyb,
            tc.tile_pool(name="psum", bufs=1, space="PSUM") as ps,
        ):
            xt = pp.tile([128, NEC, S], BF16, tag="xt")
            wqs = pp.tile([128, 4, NEC, 128], BF16, tag="wq")
            wks = pp.tile([128, 4, NEC, 128], BF16, tag="wk")
            wv = pp.tile([128, NEC, GW], BF16, tag="wv")
            wr = pp.tile([128, 4, EMB], BF16, tag="wr")
            kt = pp.tile([128, 4, S], BF16, tag="kt")
            qt = pp.tile([128, NQC, 4, QW], BF16, tag="qt")
            v = pp.tile([128, NKB, HPC, HD + 1], BF16, tag="v")
            mo = pp.tile([128, 640], BF16, tag="consts")
            mask = mo[:, 0:128]
            zeros = mo[:, 256:640]

            # ---- input DMAs, spread across engine queues ----
            nc.gpsimd.dma_start(
                wv[:], wv_d.ap().rearrange("p (c n) -> p c n", c=NEC))
            nc.gpsimd.dma_start(mo[:], cn_d.ap())
            nc.gpsimd.memset(v[:, :, :, HD], 1.0)
            # xt loads by seq-quarters: round 0 (P1 sb0-3 + P2 c0) only
            # reads cols 0:512, so it can start after ~1MB instead of 4MB
            def xt_eng(e):
                return nc.sync if e % 2 == 0 else (
                    nc.scalar if e < 6 else nc.gpsimd)

            for e in range(NEC):
                xt_eng(e).dma_start(
                    xt[:, e, 0:QW], xt_d.ap()[e * 128:(e + 1) * 128, 0:QW])
            nc.sync.dma_start(wqs[:], wq_d.ap())
            nc.scalar.dma_start(wks[:], wk_d.ap())
            # rest of xt (cols 512:2048) AFTER the weights so it does not
            # steal HBM bandwidth from the round-0 critical path
            for e in range(NEC):
                nc.sync.dma_start(
                    xt[:, e, QW:], xt_d.ap()[e * 128:(e + 1) * 128, QW:])
            nc.gpsimd.dma_start(
                wr[:], wr_d.ap().rearrange("p (c n) -> p c n", c=4))

            # PSUM tags (8 banks total):
            #   scA [128,3,QW] x1 = 3, scB [128,2,QW] x1 = 2,
            #   acc [128,QW] x2 = 2, y [128,QW] x1 = 1
            def sc_tile(tag):
                if tag == "A":
                    return ps.tile([128, 3, QW], F32, tag="scA", bufs=1, name="scA")
                return ps.tile([128, 2, QW], F32, tag="scB", bufs=1, name="scB")

            proj_tag = ["A"]

            def proj_group(emit_mms, copy_out):
                t = sc_tile(proj_tag[0])[:, 0, :]
                proj_tag[0] = "B" if proj_tag[0] == "A" else "A"
                emit_mms(t)
                copy_out(t)
                group_tick()

            # ---- deferred attn@V + output-projection machinery ----
            pend = deque()      # (at_t, blocks, qc, h, acc)
            p4_units = deque()  # (ready_tick, qc, sbl)
            p4_late = deque()   # (qc, sbl): deferred into qc3's stream
            epi_muls = deque()  # (ready_tick, closure): deferred final scale
            cur_qc = [0]
            tick = [0]
            outts = {}

            def emit_av():
                at_t, blocks, qc, h, acc = pend.popleft()
                kbmax = 4 * (qc + 1)
                for j, kb in enumerate(blocks):
                    nc.tensor.matmul(
                        acc[0:HD + 1, :], v[:, kb, h, :], at_t[:, j, :],
                        start=(kb == 0), stop=(kb == kbmax - 1),
                    )
                if blocks[-1] == kbmax - 1:
                    emit_epilogue(qc, h, acc)

            def emit_epilogue(qc, h, acc):
                hp, ho = h // 2, (h % 2) * HD
                num = pnum.tile([HD, QW], F32, tag="num", name="num")
                nc.vector.tensor_copy(num[:], acc[0:HD, :])
                den = prc.tile([1, QW], F32, tag="den", name="den")
                nc.vector.tensor_copy(den[0:1, :], acc[HD:HD + 1, :])
                rec = prc.tile([1, QW], F32, tag="rec", name="rec")
                nc.vector.reciprocal_approx_fast(rec[0:1, :], den[0:1, :])
                bc = pbc.tile([HD, QW], F32, tag="bc", name="bc")
                nc.gpsimd.partition_broadcast(bc[:], rec[0:1, :])
                nc.vector.tensor_mul(
                    outts[qc][ho:ho + HD, hp, :], num[:], bc[:])
                if h == HPC - 1:
                    if ot_d is not None:
                        nc.sync.dma_start(ot_d.ap()[qc], outts[qc][:])
                    if qc < 3:
                        for sbl in range(4):
                            p4_late.append((qc, sbl))
                    else:
                        for sbl in range(4):
                            p4_units.append((tick[0] + 2, qc, sbl))

            def emit_p4_unit(qc, sbl, alt=False):
                outtc = outts[qc]
                ysb = pyb.tile([128, EMB], F32, tag="ysb", name="ysb")
                for ncol in range(2):
                    # in the final drain the attention acc ring is free; use
                    # it as a second buffer so mm groups don't wait on copies
                    ytag = "acc" if (alt and ncol == 1) else "y"
                    ybufs = 2 if ytag == "acc" else 1
                    yps = ps.tile([128, QW], F32, tag=ytag, bufs=ybufs,
                                  name="yps")
                    for hp in range(4):
                        nc.tensor.matmul(
                            yps[:],
                            outtc[:, hp, sbl * 128:(sbl + 1) * 128],
                            wr[:, hp, ncol * QW:(ncol + 1) * QW],
                            start=(hp == 0), stop=(hp == 3),
                        )
                    nc.vector.tensor_copy(
                        ysb[:, ncol * QW:(ncol + 1) * QW], yps[:])
                sb = qc * 4 + sbl
                nc.sync.dma_start(y_d.ap()[sb * 128:(sb + 1) * 128, :], ysb[:])

            def group_tick():
                tick[0] += 1
                while epi_muls and epi_muls[0][0] <= tick[0]:
                    epi_muls.popleft()[1]()
                if len(pend) >= 4:
                    emit_av()
                if cur_qc[0] == 3 and p4_late and tick[0] % 4 == 0:
                    qc, sbl = p4_late.popleft()
                    emit_p4_unit(qc, sbl)
                if p4_units and p4_units[0][0] <= tick[0]:
                    _, qc, sbl = p4_units.popleft()
                    emit_p4_unit(qc, sbl)

            # ---- projection work units ----
            def emit_p1_unit(sb):
                def mms(t):
                    for e in range(NEC):
                        nc.tensor.matmul(
                            t, xt[:, e, sb * 128:(sb + 1) * 128], wv[:, e, :],
                            start=(e == 0), stop=(e == NEC - 1),
                        )

                def cpy(t):
                    nc.vector.tensor_copy(
                        v[:, sb, :, 0:HD],
                        t.rearrange("p (h d) -> p h d", d=HD),
                    )
                proj_group(mms, cpy)

            def emit_p2_unit(c, wsrc, is_q, hp):
                csl = slice(c * QW, (c + 1) * QW)

                def mms(t):
                    for e in range(NEC):
                        nc.tensor.matmul(
                            t, wsrc[:, hp, e, :], xt[:, e, csl],
                            start=(e == 0), stop=(e == NEC - 1),
                        )

                def cpy(t):
                    if is_q:
                        nc.vector.tensor_copy(qt[:, c, hp, :], t)
                    else:
                        nc.vector.tensor_copy(kt[:, hp, csl], t)
                proj_group(mms, cpy)

            def proj_round_units(rnd):
                units = []
                p1_sbs = range(4 * rnd, 4 * rnd + 4)
                for sb in p1_sbs:
                    units.append(lambda sb=sb: emit_p1_unit(sb))
                for wsrc, is_q in ((wqs, True), (wks, False)):
                    for hp in range(4):
                        units.append(
                            lambda wsrc=wsrc, is_q=is_q, hp=hp:
                            emit_p2_unit(rnd, wsrc, is_q, hp))
                return units

            # round 0: stream P1 (v for sb 0-7) over arriving xt chunks using
            # all 8 PSUM banks as concurrent accumulators, then P2(c=0).
            tA = ps.tile([128, 3, QW], F32, tag="scA", bufs=1, name="scA")
            tB = ps.tile([128, 2, QW], F32, tag="scB", bufs=1, name="scB")
            accs0 = [tA[:, 0, :], tA[:, 1, :], tA[:, 2, :], tB[:, 0, :]]
            for e in range(NEC):
                for i, sb in enumerate(range(4)):
                    nc.tensor.matmul(
                        accs0[i], xt[:, e, sb * 128:(sb + 1) * 128],
                        wv[:, e, :],
                        start=(e == 0), stop=(e == NEC - 1),
                    )
            for i, sb in enumerate(range(4)):
                nc.vector.tensor_copy(
                    v[:, sb, :, 0:HD],
                    accs0[i].rearrange("p (h d) -> p h d", d=HD),
                )
            for u in proj_round_units(0)[4:]:
                u()
            PUNITS_PER_HEAD = [2, 2, 2, 2, 1, 1, 1, 1]

            for rnd in range(4):
                # attention for query chunk qc = rnd
                qc = rnd
                cur_qc[0] = qc
                punits = deque(proj_round_units(rnd + 1)) if rnd < 3 else deque()
                outts[qc] = po.tile([128, 4, QW], BF16, tag="outt", name="outt")
                for h in range(HPC):
                    hp, ho = h // 2, (h % 2) * HD
                    acc = ps.tile([128, QW], F32, tag="acc", bufs=2, name="acc")
                    for gtag, blocks in _groups_for(qc):
                        n = len(blocks)
                        sc_t = sc_tile(gtag)
                        for j, kb in enumerate(blocks):
                            q0 = max(0, (kb - 4 * qc)) * 128
                            nc.tensor.matmul(
                                sc_t[:, j, q0:],
                                kt[ho:ho + HD, hp, kb * 128:(kb + 1) * 128],
                                qt[ho:ho + HD, qc, hp, q0:],
                                start=True, stop=True,
                            )
                        at_t = pa.tile([128, 3, QW], BF16, tag="at", name="at")
                        nc.scalar.activation(
                            at_t[:, 0:n, :], sc_t[:, 0:n, :], EXP)
                        for j, kb in enumerate(blocks):
                            jj = kb - 4 * qc
                            if jj >= 0:  # diagonal block: causal mask
                                if jj > 0:
                                    nc.vector.tensor_copy(
                                        at_t[:, j, 0:jj * 128],
                                        zeros[:, 0:jj * 128])
                                nc.vector.tensor_mul(
                                    at_t[:, j, jj * 128:(jj + 1) * 128],
                                    at_t[:, j, jj * 128:(jj + 1) * 128],
                                    mask,
                                )
                        if ot_d is not None and qc == 0 and h == 0:
                            gi = 0 if blocks[0] == 0 else 1
                            nc.sync.dma_start(at_d.ap()[gi], at_t[:])
                        pend.append((at_t, blocks, qc, h, acc))
                        group_tick()
                    for _ in range(PUNITS_PER_HEAD[h]):
                        if punits:
                            punits.popleft()()
                    if h == HPC - 1:
                        while punits:
                            punits.popleft()()

            # ---- drain ----
            while pend:
                emit_av()
                tick[0] += 1
                while epi_muls and epi_muls[0][0] <= tick[0]:
                    epi_muls.popleft()[1]()
                if p4_late:
                    qc, sbl = p4_late.popleft()
                    emit_p4_unit(qc, sbl)
            while epi_muls:
                epi_muls.popleft()[1]()
            while p4_late:
                qc, sbl = p4_late.popleft()
                emit_p4_unit(qc, sbl)
            while p4_units:
                _, qc, sbl = p4_units.popleft()
                emit_p4_unit(qc, sbl, alt=True)

    nc.compile()
    return nc


_NC_CACHE = None


def _get_nc():
    global _NC_CACHE
    if _NC_CACHE is None:
        _NC_CACHE = build()
    return _NC_CACHE


def make_in_maps(x, Wq, Wk, Wv, Wr):
    x = np.asarray(x, dtype=np.float32)
    Wq = np.asarray(Wq, dtype=np.float32)
    Wk = np.asarray(Wk, dtype=np.float32)
    Wv = np.asarray(Wv, dtype=np.float32)
    Wr = np.asarray(Wr, dtype=np.float32)

    consts = np.zeros((128, 640), dtype=np.float32)
    consts[:, 0:128] = np.triu(np.ones((128, 128), dtype=np.float32))
    consts[:, 128:256] = 1.0
    consts = consts.astype(bfloat16)

    def swz(w):  # [1024, 512] -> [p, hp, e, n]
        return np.ascontiguousarray(
            w.reshape(NEC, 128, 4, 128).transpose(1, 2, 0, 3).astype(bfloat16))

    in_maps = []
    for core in range(NCORES):
        b, g = divmod(core, 2)
        hs = slice(g * GW, (g + 1) * GW)
        in_maps.append({
            "xt": np.ascontiguousarray(x[b].T.astype(bfloat16)),
            "wq": swz(Wq[:, hs] * SCALE),
            "wk": swz(Wk[:, hs]),
            "wv": np.ascontiguousarray(
                Wv[:, hs].reshape(NEC, 128, GW).transpose(1, 0, 2)
                .reshape(128, NEC * GW).astype(bfloat16)),
            "wr": np.ascontiguousarray(
                Wr[hs, :].reshape(4, 128, EMB).transpose(1, 0, 2)
                .reshape(128, 4 * EMB).astype(bfloat16)),
            "consts": consts,
        })
    return in_maps


def kernel(x, Wq, Wk, Wv, Wr):
    in_maps = make_in_maps(x, Wq, Wk, Wv, Wr)
    nc = _get_nc()
    res = run_bass_kernel_spmd(nc, in_maps, core_ids=list(range(NCORES)))

    y = np.empty((B, S, EMB), dtype=np.float32)
    for b in range(B):
        y[b] = res.results[2 * b]["y"] + res.results[2 * b + 1]["y"]
    return y
